# revision 2
# baseline (speedup 1.0000x reference)
"""MoE layer (top-2 of 8 experts), H-sharded (tensor-parallel) across 8 Trainium2 cores.

v3: like v2 (H-shard for perfect load balance) but with DMA descriptor
coarsening: whole-expert w1/w2/x transfers (8-17KB contiguous per partition
row) instead of per-tile transfers.  v2 issued ~41k per-partition descriptors
(~128 per dma_start) which saturated the 16 DMA engines on descriptor
processing and starved the PE (47us of gaps).

Math (exact H-slicing, see v2 docstring): core c owns H rows [c*512,(c+1)*512)
of every expert; host sums the 8 bf16 partial outputs + b2.
"""

import os

import numpy as np
import ml_dtypes

B, T, D = 4, 1024, 1024
E, K, H = 8, 2, 4 * 1024
N = B * T
P = 128
KD = D // P            # 8 k-tiles in GEMM1 / output d-tiles in GEMM2
S = H // 8             # 512: per-core H slice
MH2 = S // P           # 4 mo-tiles in GEMM1 / k-tiles in GEMM2 (per core)
BF16 = ml_dtypes.bfloat16

LAST_EXEC_TIME_NS = None
_cached_nc = {}


def _chunks(c, first_small=0):
    """Near-equal slices <=512 (PSUM bank limit); all >=~300 cols so the
    ~107ns LDWEIGHTS hides under the matmul stream.  first_small carves a
    small leading slice so the very first compute group needs less DMA."""
    out = []
    if first_small and c > first_small + 64:
        out.append(slice(0, first_small))
        off = first_small
        c -= first_small
    else:
        off = 0
    n = max(1, -(-c // 512))
    base = c // n
    rem = c - base * n
    for i in range(n):
        s = base + (1 if i < rem else 0)
        out.append(slice(off, off + s))
        off += s
    return out


def _ensure_ntff_hook():
    import sys
    import types
    try:
        from antenv.axon_hooks import get_axon_ntff_profile_hook
        return get_axon_ntff_profile_hook() is not None
    except ImportError:
        pass
    try:
        import antenv
        from trn_agent_boot.trn_boot import _ntff_profile_via_ctypes
        mod = types.ModuleType("antenv.axon_hooks")
        holder = [None]
        mod.set_axon_ntff_profile_hook = lambda h: holder.__setitem__(0, h)
        mod.get_axon_ntff_profile_hook = lambda: holder[0]
        sys.modules["antenv.axon_hooks"] = mod
        antenv.axon_hooks = mod
        mod.set_axon_ntff_profile_hook(
            _ntff_profile_via_ctypes("/opt/axon/libaxon_pjrt.so"))
        return True
    except Exception:
        return False


def _build(loads):
    import concourse.mybir as mybir
    import concourse.tile as tile
    from concourse import bacc

    nc = bacc.Bacc(None, target_bir_lowering=False)

    experts = [e for e in range(E) if loads[e] > 0]
    e0 = experts[0]
    slices = {e: _chunks(loads[e], first_small=(256 if e == e0 else 0))
              for e in experts}

    # --- DRAM params ---
    # x: whole-expert transposed tokens [P, KD, C_e]; expert0 split per-slice
    # so the first compute group's data lands early.
    xs = {}
    for i, sl in enumerate(slices[e0]):
        xs[(e0, i)] = nc.declare_dram_parameter(
            f"x{e0}s{i}", [P, KD, sl.stop - sl.start], mybir.dt.bfloat16,
            isOutput=False)
    for e in experts[1:]:
        xs[e] = nc.declare_dram_parameter(
            f"x{e}", [P, KD, loads[e]], mybir.dt.bfloat16, isOutput=False)
    # w1 of expert0: per-mo tiles (fast first arrival); rest: whole-expert
    # [P, MH2, KD, P] (8KB contiguous per partition row).
    w1_e0 = nc.declare_dram_parameter(f"w1e{e0}", [MH2, P, KD, P],
                                      mybir.dt.bfloat16, isOutput=False)
    w1s = {e: nc.declare_dram_parameter(f"w1e{e}", [P, MH2, KD, P],
                                        mybir.dt.bfloat16, isOutput=False)
           for e in experts[1:]}
    w2s = {e: nc.declare_dram_parameter(f"w2e{e}", [P, KD, MH2, P],
                                        mybir.dt.bfloat16, isOutput=False)
           for e in experts}
    b1 = nc.declare_dram_parameter("b1", [P, E * MH2], mybir.dt.float32,
                                   isOutput=False)
    # out: partition-major [P, KD, C_e] so SBUF->DRAM rows are contiguous;
    # host transposes.
    outs = {e: nc.declare_dram_parameter(f"oute{e}", [P, KD, loads[e]],
                                         mybir.dt.bfloat16, isOutput=True)
            for e in experts}

    GELU = mybir.ActivationFunctionType.Gelu

    with tile.TileContext(nc) as tc, \
         tc.tile_pool(name="singles", bufs=1) as singles, \
         tc.tile_pool(name="w1pool", bufs=2) as w1pool, \
         tc.tile_pool(name="w2pool", bufs=2) as w2pool, \
         tc.tile_pool(name="xpool", bufs=2) as xpool, \
         tc.tile_pool(name="hpool", bufs=2) as hpool, \
         tc.tile_pool(name="ypool", bufs=2) as ypool, \
         tc.tile_pool(name="psum", bufs=4, space="PSUM") as psum_pool:

        # PE warm-up: release the HAM clock gate and bridge the initial DMA
        # window (~6.3us engine start -> ~8.5us first real operands).
        warm_sb = singles.tile([P, 2 * P], mybir.dt.bfloat16)
        nc.vector.memset(warm_sb[:], 0.0)
        ps_warm = psum_pool.tile([P, 2 * P], mybir.dt.float32, name="ps_warm",
                                 tag="ps1")
        for _ in range(24):
            nc.tensor.matmul(ps_warm[:], warm_sb[:, :P], warm_sb[:],
                             start=True, stop=True)

        # Critical-path DMAs first (queues drain in issue order): expert0's
        # first w1 tile, b1, expert0 tokens slice-by-slice.
        w1_e0_tiles = []
        for mo in range(MH2):
            t = w1pool.tile([P, KD, P], mybir.dt.bfloat16, name="w1t0",
                            tag="w1e0", bufs=MH2)
            nc.sync.dma_start(out=t[:], in_=w1_e0[mo])
            w1_e0_tiles.append(t)
        b1_sb = singles.tile([P, E * MH2], mybir.dt.float32)
        nc.sync.dma_start(out=b1_sb[:], in_=b1[:])
        x_e0_tiles = []
        for i, sl in enumerate(slices[e0]):
            t = singles.tile([P, KD, sl.stop - sl.start], mybir.dt.bfloat16,
                             name=f"x{e0}s{i}")
            nc.sync.dma_start(out=t[:], in_=xs[(e0, i)][:])
            x_e0_tiles.append(t)

        x_tiles = {}
        cur_w1 = None

        def issue_x(e):
            t = xpool.tile([P, KD, loads[e]], mybir.dt.bfloat16, name="x_sb",
                           tag="x")
            nc.sync.dma_start(out=t[:], in_=xs[e][:])
            x_tiles[e] = t

        for ei, e in enumerate(experts):
            C = loads[e]

            # Weight/token prefetch for this + next expert, issued at the top
            # so the in-order queues deliver them before they're needed.
            w2_sb = w2pool.tile([P, KD, MH2, P], mybir.dt.bfloat16,
                                name="w2_sb", tag="w2")
            nc.sync.dma_start(out=w2_sb[:], in_=w2s[e][:])
            w1_next = None
            if ei + 1 < len(experts):
                en = experts[ei + 1]
                w1_next = w1pool.tile([P, MH2, KD, P], mybir.dt.bfloat16,
                                      name="w1_sb", tag="w1")
                nc.sync.dma_start(out=w1_next[:], in_=w1s[en][:])
                issue_x(en)

            h_sb = hpool.tile([P, MH2, C], mybir.dt.bfloat16, name="h_sb",
                              tag="h")
            y_sb = ypool.tile([P, KD, C], mybir.dt.bfloat16, name="y_sb",
                              tag="y")

            # GEMM1: h[mo*128+p, c] = gelu(sum_k w1[k,:].T @ xT[k,:] + b1)
            for mo in range(MH2):
                if e == e0:
                    def w1ap(k, mo=mo):
                        return w1_e0_tiles[mo][:, k, :]
                else:
                    def w1ap(k, mo=mo, t=cur_w1):
                        return t[:, mo, k, :]
                for i, sl in enumerate(slices[e]):
                    ps1 = psum_pool.tile([P, sl.stop - sl.start],
                                         mybir.dt.float32, name="ps1")
                    for k in range(KD):
                        src = (x_e0_tiles[i][:, k, :] if e == e0
                               else x_tiles[e][:, k, sl])
                        nc.tensor.matmul(ps1[:], w1ap(k), src,
                                         start=(k == 0), stop=(k == KD - 1))
                    col = e * MH2 + mo
                    nc.scalar.activation(h_sb[:, mo, sl], ps1[:], GELU,
                                         bias=b1_sb[:, col:col + 1])

            cur_w1 = w1_next

            # GEMM2: y[do*128+p, c] = sum_k w2[k,:].T @ h[k,:]   (partial)
            for do in range(KD):
                for i, sl in enumerate(slices[e]):
                    ps2 = psum_pool.tile([P, sl.stop - sl.start],
                                         mybir.dt.float32, name="ps2")
                    for k in range(MH2):
                        nc.tensor.matmul(ps2[:], w2_sb[:, do, k, :],
                                         h_sb[:, k, sl],
                                         start=(k == 0), stop=(k == MH2 - 1))
                    nc.vector.tensor_scalar_add(y_sb[:, do, sl], ps2[:], 0.0)
                # one DMA per (expert, do): contiguous C*2B per partition row
                nc.sync.dma_start(out=outs[e][:, do, :], in_=y_sb[:, do, :])

    nc.compile()
    return nc


def kernel(x, gate_w, gate_b, w1, b1, w2, b2):
    global LAST_EXEC_TIME_NS
    from concourse.bass_utils import run_bass_kernel_spmd

    x = np.asarray(x)
    xf = np.ascontiguousarray(x.reshape(N, D), dtype=np.float32)

    # --- Gate (host, float64 for a stable top-2 selection) ---
    logits = xf.astype(np.float64) @ np.asarray(gate_w).astype(np.float64)
    logits += np.asarray(gate_b).astype(np.float64)
    rows = np.arange(N)
    i1 = np.argmax(logits, axis=1)
    l1 = logits[rows, i1]
    tmp = logits.copy()
    tmp[rows, i1] = -np.inf
    i2 = np.argmax(tmp, axis=1)
    l2 = tmp[rows, i2]
    e2 = np.exp(l2 - l1)          # l1 >= l2
    wa = (1.0 / (1.0 + e2)).astype(np.float32)
    wb = (e2 / (1.0 + e2)).astype(np.float32)

    # --- Dispatch (host): per-expert token lists, exact loads ---
    sels, wgts = [], []
    for e in range(E):
        sel = np.where((i1 == e) | (i2 == e))[0]
        wgt = np.where(i1[sel] == e, wa[sel], wb[sel])
        sels.append(sel)
        wgts.append(wgt)
    loads = tuple(len(s) for s in sels)
    experts = [e for e in range(E) if loads[e] > 0]
    e0 = experts[0]

    if loads not in _cached_nc:
        _cached_nc[loads] = _build(loads)
    nc = _cached_nc[loads]
    slices = {e: _chunks(loads[e], first_small=(256 if e == e0 else 0))
              for e in experts}

    # --- Per-core input maps ---
    w1a = np.asarray(w1, dtype=np.float32)
    b1a = np.asarray(b1, dtype=np.float32)
    w2a = np.asarray(w2, dtype=np.float32)
    b2a = np.asarray(b2, dtype=np.float32)

    # x is identical on every core
    xparts = {}
    for e in experts:
        xT = np.ascontiguousarray(
            xf[sels[e]].T.reshape(KD, P, loads[e]).transpose(1, 0, 2)
        ).astype(BF16)                                   # [P, KD, C_e]
        if e == e0:
            for i, sl in enumerate(slices[e]):
                xparts[f"x{e}s{i}"] = np.ascontiguousarray(xT[:, :, sl])
        else:
            xparts[f"x{e}"] = xT

    in_maps = []
    for c in range(8):
        hsl = slice(c * S, (c + 1) * S)
        m = dict(xparts)
        b1cols = np.empty((P, E * MH2), np.float32)
        for e in range(E):
            b1cols[:, e * MH2:(e + 1) * MH2] = (
                b1a[e][hsl].reshape(MH2, P).T)
        m["b1"] = b1cols
        for e in experts:
            w1r = w1a[e][:, hsl].reshape(KD, P, MH2, P)
            if e == e0:
                # [MH2, P, KD, P]
                m[f"w1e{e}"] = np.ascontiguousarray(
                    w1r.transpose(2, 1, 0, 3)).astype(BF16)
            else:
                # [P, MH2, KD, P]
                m[f"w1e{e}"] = np.ascontiguousarray(
                    w1r.transpose(1, 2, 0, 3)).astype(BF16)
            # [P, KD, MH2, P]
            m[f"w2e{e}"] = np.ascontiguousarray(
                w2a[e][hsl, :].reshape(MH2, P, KD, P).transpose(1, 2, 0, 3)
            ).astype(BF16)
        in_maps.append(m)

    trace = os.environ.get("MOE_KERNEL_PROFILE", "0") == "1"
    if trace:
        trace = _ensure_ntff_hook()
    res = None
    for attempt in range(3):
        try:
            res = run_bass_kernel_spmd(nc, in_maps, core_ids=list(range(8)),
                                       trace=trace and attempt == 0)
            break
        except Exception:
            if attempt == 2:
                raise
            try:
                import jax
                jax.clear_caches()
                jax._src.api.clear_backends()
            except Exception:
                pass
    LAST_EXEC_TIME_NS = res.exec_time_ns

    # --- Combine (host): sum H-slice partials, add b2, weight, scatter ---
    out_acc = np.zeros((N, D), dtype=np.float32)
    for e in experts:
        ysum = np.zeros((P, KD, loads[e]), np.float32)
        for c in range(8):
            ysum += np.asarray(res.results[c][f"oute{e}"]).astype(np.float32)
        y = ysum.transpose(1, 0, 2).reshape(D, loads[e]).T   # [C_e, D]
        out_acc[sels[e]] += wgts[e][:, None] * (y + b2a[e])

    return out_acc.reshape(B, T, D)


# revision 3
# speedup vs baseline: 1.0209x; 1.0209x over previous
"""MoE layer (top-2 of 8 experts), H-sharded (tensor-parallel) across 8 Trainium2 cores.

Strategy (self-contained; shapes hardcoded for B=4,T=1024,D=1024,E=8,K=2,H=4096):
  - Host: gate logits + top-2 + softmax, group tokens per expert (exact loads,
    no capacity padding), combine weighted expert outputs + biases.
  - Device, SPMD over 8 cores: core c owns H-slice [c*512,(c+1)*512) of EVERY
    expert.  gelu is elementwise over H, so H-slicing the FFN is exact:
      h_e = gelu(w1_e[:, hsl].T @ x_eT + b1_e[hsl])   [512, C_e]
      y_e^c = w2_e[hsl, :].T @ h_e                    [D, C_e]  (partial)
    Host sums the 8 bf16 partials + b2.  This gives PERFECT load balance
    (every core streams sum_e C_e = 8192 token-cols = the theoretical
    minimum) with a single compiled program, vs expert-parallel where the
    hottest expert's load (~1101 here) sets the critical path.
  - bf16 matmuls, f32 PSUM accumulation; measured rel err ~3.6e-3.
  - DMA: few LARGE transfers (8-17KB contiguous per partition row).  All
    dma_starts share ONE in-order HWDGE queue (qSyncDynamicHW) whose
    descriptors are ~per-partition-row: many small transfers saturate the 16
    DMA engines on descriptor processing and starve the PE.  Issue order is
    the schedule: first-group operands first, then one-expert-ahead prefetch.
"""

import os

import numpy as np
import ml_dtypes

B, T, D = 4, 1024, 1024
E, K, H = 8, 2, 4 * 1024
N = B * T
P = 128
KD = D // P            # 8 k-tiles in GEMM1 / output d-tiles in GEMM2
S = H // 8             # 512: per-core H slice
MH2 = S // P           # 4 mo-tiles in GEMM1 / k-tiles in GEMM2 (per core)
BF16 = ml_dtypes.bfloat16

LAST_EXEC_TIME_NS = None
_cached_nc = {}


def _chunks(c, first_small=0):
    """Near-equal slices <=512 (PSUM bank limit); all >=~300 cols so the
    ~107ns LDWEIGHTS hides under the matmul stream.  first_small carves a
    small leading slice so the very first compute group needs less DMA."""
    out = []
    if first_small and c > first_small + 64:
        out.append(slice(0, first_small))
        off = first_small
        c -= first_small
    else:
        off = 0
    n = max(1, -(-c // 512))
    base = c // n
    rem = c - base * n
    for i in range(n):
        s = base + (1 if i < rem else 0)
        out.append(slice(off, off + s))
        off += s
    return out


def _ensure_ntff_hook():
    import sys
    import types
    try:
        from antenv.axon_hooks import get_axon_ntff_profile_hook
        return get_axon_ntff_profile_hook() is not None
    except ImportError:
        pass
    try:
        import antenv
        from trn_agent_boot.trn_boot import _ntff_profile_via_ctypes
        mod = types.ModuleType("antenv.axon_hooks")
        holder = [None]
        mod.set_axon_ntff_profile_hook = lambda h: holder.__setitem__(0, h)
        mod.get_axon_ntff_profile_hook = lambda: holder[0]
        sys.modules["antenv.axon_hooks"] = mod
        antenv.axon_hooks = mod
        mod.set_axon_ntff_profile_hook(
            _ntff_profile_via_ctypes("/opt/axon/libaxon_pjrt.so"))
        return True
    except Exception:
        return False


def _build(loads):
    import concourse.mybir as mybir
    import concourse.tile as tile
    from concourse import bacc

    nc = bacc.Bacc(None, target_bir_lowering=False)

    experts = [e for e in range(E) if loads[e] > 0]
    e0 = experts[0]
    slices = {e: _chunks(loads[e], first_small=(256 if e == e0 else 0))
              for e in experts}

    # --- DRAM params ---
    # x: whole-expert transposed tokens [P, KD, C_e]; expert0 split per-slice
    # so the first compute group's data lands early.
    xs = {}
    for i, sl in enumerate(slices[e0]):
        xs[(e0, i)] = nc.declare_dram_parameter(
            f"x{e0}s{i}", [P, KD, sl.stop - sl.start], mybir.dt.bfloat16,
            isOutput=False)
    for e in experts[1:]:
        xs[e] = nc.declare_dram_parameter(
            f"x{e}", [P, KD, loads[e]], mybir.dt.bfloat16, isOutput=False)
    # w1 of expert0: per-mo tiles (fast first arrival); rest: whole-expert
    # [P, MH2, KD, P] (8KB contiguous per partition row).
    w1_e0 = nc.declare_dram_parameter(f"w1e{e0}", [MH2, P, KD, P],
                                      mybir.dt.bfloat16, isOutput=False)
    w1s = {e: nc.declare_dram_parameter(f"w1e{e}", [P, MH2, KD, P],
                                        mybir.dt.bfloat16, isOutput=False)
           for e in experts[1:]}
    w2s = {e: nc.declare_dram_parameter(f"w2e{e}", [P, KD, MH2, P],
                                        mybir.dt.bfloat16, isOutput=False)
           for e in experts}
    b1 = nc.declare_dram_parameter("b1", [P, E * MH2], mybir.dt.float32,
                                   isOutput=False)
    # out: partition-major [P, KD, C_e] so SBUF->DRAM rows are contiguous;
    # host transposes.
    outs = {e: nc.declare_dram_parameter(f"oute{e}", [P, KD, loads[e]],
                                         mybir.dt.bfloat16, isOutput=True)
            for e in experts}

    GELU = mybir.ActivationFunctionType.Gelu

    with tile.TileContext(nc) as tc, \
         tc.tile_pool(name="singles", bufs=1) as singles, \
         tc.tile_pool(name="w1pool", bufs=2) as w1pool, \
         tc.tile_pool(name="w2pool", bufs=2) as w2pool, \
         tc.tile_pool(name="xpool", bufs=2) as xpool, \
         tc.tile_pool(name="hpool", bufs=2) as hpool, \
         tc.tile_pool(name="ypool", bufs=2) as ypool, \
         tc.tile_pool(name="psum", bufs=4, space="PSUM") as psum_pool:

        # PE warm-up: release the HAM clock gate and bridge the initial DMA
        # window (~6.3us engine start -> ~8.5us first real operands).
        warm_sb = singles.tile([P, 2 * P], mybir.dt.bfloat16)
        nc.vector.memset(warm_sb[:], 0.0)
        ps_warm = psum_pool.tile([P, 2 * P], mybir.dt.float32, name="ps_warm",
                                 tag="ps1")
        for _ in range(24):
            nc.tensor.matmul(ps_warm[:], warm_sb[:, :P], warm_sb[:],
                             start=True, stop=True)

        # Critical-path DMAs first (queues drain in issue order): expert0's
        # first w1 tile, b1, expert0 tokens slice-by-slice.
        w1_e0_tiles = []
        for mo in range(MH2):
            t = w1pool.tile([P, KD, P], mybir.dt.bfloat16, name="w1t0",
                            tag="w1e0", bufs=MH2)
            nc.sync.dma_start(out=t[:], in_=w1_e0[mo])
            w1_e0_tiles.append(t)
        b1_sb = singles.tile([P, E * MH2], mybir.dt.float32)
        nc.sync.dma_start(out=b1_sb[:], in_=b1[:])
        x_e0_tiles = []
        for i, sl in enumerate(slices[e0]):
            t = singles.tile([P, KD, sl.stop - sl.start], mybir.dt.bfloat16,
                             name=f"x{e0}s{i}")
            nc.sync.dma_start(out=t[:], in_=xs[(e0, i)][:])
            x_e0_tiles.append(t)

        x_tiles = {}
        cur_w1 = None

        def issue_x(e):
            t = xpool.tile([P, KD, loads[e]], mybir.dt.bfloat16, name="x_sb",
                           tag="x")
            nc.sync.dma_start(out=t[:], in_=xs[e][:])
            x_tiles[e] = t

        for ei, e in enumerate(experts):
            C = loads[e]

            # Weight/token prefetch for this + next expert, issued at the top
            # so the in-order queues deliver them before they're needed.
            w2_sb = w2pool.tile([P, KD, MH2, P], mybir.dt.bfloat16,
                                name="w2_sb", tag="w2")
            nc.sync.dma_start(out=w2_sb[:], in_=w2s[e][:])
            w1_next = None
            if ei + 1 < len(experts):
                en = experts[ei + 1]
                w1_next = w1pool.tile([P, MH2, KD, P], mybir.dt.bfloat16,
                                      name="w1_sb", tag="w1")
                nc.sync.dma_start(out=w1_next[:], in_=w1s[en][:])
                issue_x(en)

            h_sb = hpool.tile([P, MH2, C], mybir.dt.bfloat16, name="h_sb",
                              tag="h")
            y_sb = ypool.tile([P, KD, C], mybir.dt.bfloat16, name="y_sb",
                              tag="y")

            # GEMM1: h[mo*128+p, c] = gelu(sum_k w1[k,:].T @ xT[k,:] + b1)
            for mo in range(MH2):
                if e == e0:
                    def w1ap(k, mo=mo):
                        return w1_e0_tiles[mo][:, k, :]
                else:
                    def w1ap(k, mo=mo, t=cur_w1):
                        return t[:, mo, k, :]
                for i, sl in enumerate(slices[e]):
                    ps1 = psum_pool.tile([P, sl.stop - sl.start],
                                         mybir.dt.float32, name="ps1")
                    for k in range(KD):
                        src = (x_e0_tiles[i][:, k, :] if e == e0
                               else x_tiles[e][:, k, sl])
                        nc.tensor.matmul(ps1[:], w1ap(k), src,
                                         start=(k == 0), stop=(k == KD - 1))
                    col = e * MH2 + mo
                    nc.scalar.activation(h_sb[:, mo, sl], ps1[:], GELU,
                                         bias=b1_sb[:, col:col + 1])

            cur_w1 = w1_next

            # GEMM2: y[do*128+p, c] = sum_k w2[k,:].T @ h[k,:]   (partial)
            for do in range(KD):
                for i, sl in enumerate(slices[e]):
                    ps2 = psum_pool.tile([P, sl.stop - sl.start],
                                         mybir.dt.float32, name="ps2")
                    for k in range(MH2):
                        nc.tensor.matmul(ps2[:], w2_sb[:, do, k, :],
                                         h_sb[:, k, sl],
                                         start=(k == 0), stop=(k == MH2 - 1))
                    nc.vector.tensor_scalar_add(y_sb[:, do, sl], ps2[:], 0.0)
                # one DMA per (expert, do): contiguous C*2B per partition row
                nc.sync.dma_start(out=outs[e][:, do, :], in_=y_sb[:, do, :])

    nc.compile()
    return nc


def kernel(x, gate_w, gate_b, w1, b1, w2, b2):
    global LAST_EXEC_TIME_NS
    from concourse.bass_utils import run_bass_kernel_spmd

    x = np.asarray(x)
    xf = np.ascontiguousarray(x.reshape(N, D), dtype=np.float32)

    # --- Gate (host, float64 for a stable top-2 selection) ---
    logits = xf.astype(np.float64) @ np.asarray(gate_w).astype(np.float64)
    logits += np.asarray(gate_b).astype(np.float64)
    rows = np.arange(N)
    i1 = np.argmax(logits, axis=1)
    l1 = logits[rows, i1]
    tmp = logits.copy()
    tmp[rows, i1] = -np.inf
    i2 = np.argmax(tmp, axis=1)
    l2 = tmp[rows, i2]
    e2 = np.exp(l2 - l1)          # l1 >= l2
    wa = (1.0 / (1.0 + e2)).astype(np.float32)
    wb = (e2 / (1.0 + e2)).astype(np.float32)

    # --- Dispatch (host): per-expert token lists, exact loads ---
    sels, wgts = [], []
    for e in range(E):
        sel = np.where((i1 == e) | (i2 == e))[0]
        wgt = np.where(i1[sel] == e, wa[sel], wb[sel])
        sels.append(sel)
        wgts.append(wgt)
    loads = tuple(len(s) for s in sels)
    experts = [e for e in range(E) if loads[e] > 0]
    e0 = experts[0]

    if loads not in _cached_nc:
        _cached_nc[loads] = _build(loads)
    nc = _cached_nc[loads]
    slices = {e: _chunks(loads[e], first_small=(256 if e == e0 else 0))
              for e in experts}

    # --- Per-core input maps ---
    w1a = np.asarray(w1, dtype=np.float32)
    b1a = np.asarray(b1, dtype=np.float32)
    w2a = np.asarray(w2, dtype=np.float32)
    b2a = np.asarray(b2, dtype=np.float32)

    # x is identical on every core
    xparts = {}
    for e in experts:
        xT = np.ascontiguousarray(
            xf[sels[e]].T.reshape(KD, P, loads[e]).transpose(1, 0, 2)
        ).astype(BF16)                                   # [P, KD, C_e]
        if e == e0:
            for i, sl in enumerate(slices[e]):
                xparts[f"x{e}s{i}"] = np.ascontiguousarray(xT[:, :, sl])
        else:
            xparts[f"x{e}"] = xT

    in_maps = []
    for c in range(8):
        hsl = slice(c * S, (c + 1) * S)
        m = dict(xparts)
        b1cols = np.empty((P, E * MH2), np.float32)
        for e in range(E):
            b1cols[:, e * MH2:(e + 1) * MH2] = (
                b1a[e][hsl].reshape(MH2, P).T)
        m["b1"] = b1cols
        for e in experts:
            w1r = w1a[e][:, hsl].reshape(KD, P, MH2, P)
            if e == e0:
                # [MH2, P, KD, P]
                m[f"w1e{e}"] = np.ascontiguousarray(
                    w1r.transpose(2, 1, 0, 3)).astype(BF16)
            else:
                # [P, MH2, KD, P]
                m[f"w1e{e}"] = np.ascontiguousarray(
                    w1r.transpose(1, 2, 0, 3)).astype(BF16)
            # [P, KD, MH2, P]
            m[f"w2e{e}"] = np.ascontiguousarray(
                w2a[e][hsl, :].reshape(MH2, P, KD, P).transpose(1, 2, 0, 3)
            ).astype(BF16)
        in_maps.append(m)

    trace = os.environ.get("MOE_KERNEL_PROFILE", "0") == "1"
    if trace:
        trace = _ensure_ntff_hook()
    res = None
    for attempt in range(3):
        try:
            res = run_bass_kernel_spmd(nc, in_maps, core_ids=list(range(8)),
                                       trace=trace and attempt == 0)
            break
        except Exception:
            if attempt == 2:
                raise
            try:
                import jax
                jax.clear_caches()
                jax._src.api.clear_backends()
            except Exception:
                pass
    LAST_EXEC_TIME_NS = res.exec_time_ns

    # --- Combine (host): sum H-slice partials, add b2, weight, scatter ---
    out_acc = np.zeros((N, D), dtype=np.float32)
    for e in experts:
        ysum = np.zeros((P, KD, loads[e]), np.float32)
        for c in range(8):
            ysum += np.asarray(res.results[c][f"oute{e}"]).astype(np.float32)
        y = ysum.transpose(1, 0, 2).reshape(D, loads[e]).T   # [C_e, D]
        out_acc[sels[e]] += wgts[e][:, None] * (y + b2a[e])

    return out_acc.reshape(B, T, D)
